# revision 22
# baseline (speedup 1.0000x reference)
"""GQA attention kernel for 8 TRN2 NeuronCores (Bass/Tile, SPMD).

Sharding: core c -> (batch b = c // 4, kv-head kv = c % 4). Each core computes
the 4 query heads of its kv group for its batch and a partial (transposed)
output projection; the host sums the 4 partials per batch.

All device matmuls run in float32r (fp32 bits, full-rate PE path at free-dim
>= 256). Attention is computed in transposed layout throughout:
  QT/KT [hd, t]  ->  S.T [k, q] psum  ->  exp on ACT  ->  P.T [k, q]
  O.T [hd, q] = V[k, hd].T-accumulated PV matmuls
Softmax denominators come from a ones-vector matmul accumulated alongside PV;
normalization is a gpsimd partition-broadcast of 1/l followed by a DVE mul.
RoPE's rotate-half is a +-1 permutation matmul on the hd (partition) axis.
"""

import os
import sys

import numpy as np

for _p in ("/opt/trn_rl_repo", "/root/.axon_site/_ro/trn_rl_repo"):
    if os.path.isdir(_p) and _p not in sys.path:
        sys.path.insert(0, _p)

import concourse.bass as bass  # noqa: E402
import concourse.mybir as mybir  # noqa: E402
from concourse import bacc  # noqa: E402
from concourse.tile import TileContext  # noqa: E402
from concourse.bass_utils import run_bass_kernel_spmd  # noqa: E402

B, T, D = 2, 2048, 2048
H, HKV, HD = 16, 4, 128
G = H // HKV            # query heads per kv head (= per core)
EQ = G * HD             # 512: query-projection rows per core
P = 128
TC = 512                # t-chunk (free dim of every matmul)
NJ = T // TC            # 4 chunks
DT = D // P             # 16 contraction tiles
SCALE = 1.0 / float(np.sqrt(HD))

F32 = mybir.dt.float32
F32R = mybir.dt.float32r
EXP = mybir.ActivationFunctionType.Exp

_CACHE = {}


def _build():
    nc = bacc.Bacc("TRN2", target_bir_lowering=False, debug=False)

    xT = nc.declare_dram_parameter("xT", [D, T], F32R, isOutput=False)
    wqT = nc.declare_dram_parameter("wqT", [D, EQ], F32R, isOutput=False)
    wkT = nc.declare_dram_parameter("wkT", [D, HD], F32R, isOutput=False)
    wvT = nc.declare_dram_parameter("wvT", [D, HD], F32R, isOutput=False)
    woT = nc.declare_dram_parameter("woT", [EQ, D], F32R, isOutput=False)
    cosT = nc.declare_dram_parameter("cosT", [HD, T], F32R, isOutput=False)
    sinT = nc.declare_dram_parameter("sinT", [HD, T], F32R, isOutput=False)
    rmat = nc.declare_dram_parameter("rmat", [HD, HD], F32R, isOutput=False)
    iden = nc.declare_dram_parameter("iden", [P, P], F32R, isOutput=False)
    masks = nc.declare_dram_parameter("masks", [P, G, TC], F32R, isOutput=False)
    ones_k = nc.declare_dram_parameter("ones_k", [P, 1], F32R, isOutput=False)
    yT = nc.declare_dram_parameter("yT", [D, T], F32, isOutput=True)

    with TileContext(nc) as tc:
        with (
            tc.tile_pool(name="const", bufs=1) as cst,
            tc.tile_pool(name="kv", bufs=1) as kvp,
            tc.tile_pool(name="ot", bufs=1) as otp,
        ):
            # Constants ride the gpsimd SWDGE ring so they don't delay the
            # weight/x loads on the two HWDGE rings.
            cos_sb = cst.tile([HD, T], F32R, tag="cos")
            sin_sb = cst.tile([HD, T], F32R, tag="sin")
            rmat_sb = cst.tile([HD, HD], F32R, tag="rmat")
            iden_sb = cst.tile([P, P], F32R, tag="iden")
            mask_sb = cst.tile([P, G, TC], F32R, tag="mask")
            onek_sb = cst.tile([P, 1], F32R, tag="onek")
            nc.gpsimd.dma_start(cos_sb[:], cosT[:])
            nc.gpsimd.dma_start(sin_sb[:], sinT[:])
            nc.gpsimd.dma_start(rmat_sb[:], rmat[:])
            nc.gpsimd.dma_start(iden_sb[:], iden[:])
            nc.gpsimd.dma_start(mask_sb[:], masks[:])
            nc.gpsimd.dma_start(onek_sb[:], ones_k[:])

            kt_sb = kvp.tile([HD, T], F32R, tag="kt")
            v_sb = kvp.tile([P, DT, HD], F32R, tag="v")
            otn = otp.tile([HD, G, T], F32R, tag="otn")

            with (
                tc.tile_pool(name="wts", bufs=1) as wts,
                tc.tile_pool(name="xs", bufs=1) as xs,
                tc.tile_pool(name="qk", bufs=2) as qk,
                tc.tile_pool(name="work", bufs=5) as wk,
                tc.tile_pool(name="rtmp", bufs=2) as rtmp,
                tc.tile_pool(name="vt", bufs=2) as vtp,
                tc.tile_pool(name="small", bufs=2) as sml,
                tc.tile_pool(name="ps_acc", bufs=2, space="PSUM") as ps_acc,
                tc.tile_pool(name="ps_s", bufs=2, space="PSUM") as ps_s,
                tc.tile_pool(name="ps_o", bufs=2, space="PSUM") as ps_o,
                tc.tile_pool(name="ps_lb", bufs=2, space="PSUM") as ps_lb,
            ):
                # Weights ride the scalar HWDGE ring; x-chunks ride the sync
                # ring. wq is loaded in dt-quarters so the Q chains can start
                # before the full 4 MiB lands. Chain order (V,K,Q0..Q3)
                # matches DMA arrival order on each ring.
                wq_sb = wts.tile([P, DT, EQ], F32R, tag="wq")
                wk_sb = wts.tile([P, DT, HD], F32R, tag="wk")
                wv_sb = wts.tile([P, DT, HD], F32R, tag="wv")
                wqT_r = wqT.rearrange("(dt p) e -> p dt e", p=P)
                xT_r = xT.rearrange("(dt p) t -> p dt t", p=P)

                def load_x_quarter(j, q):
                    xq = xs.tile([P, 4, TC], F32R, tag=f"xc{q}", name=f"xc{q}")
                    nc.sync.dma_start(xq[:], xT_r[:, 4 * q:4 * q + 4,
                                                  j * TC:(j + 1) * TC])
                    return xq

                # One HWDGE ring executes dma_starts FIFO, so emit the loads
                # in the exact order phase A consumes them: V weights, first x
                # quarter, K weights, then alternating x quarters / wq slices.
                nc.sync.dma_start(wv_sb[:], wvT.rearrange("(dt p) e -> p dt e", p=P))
                xcq0 = [load_x_quarter(0, 0)]
                nc.sync.dma_start(wk_sb[:], wkT.rearrange("(dt p) e -> p dt e", p=P))
                xcq0.append(load_x_quarter(0, 1))
                for q in range(4):
                    nc.sync.dma_start(wq_sb[:, 4 * q:4 * q + 4],
                                      wqT_r[:, 4 * q:4 * q + 4])
                    if q < 2:
                        xcq0.append(load_x_quarter(0, q + 2))

                _pools = {"acc": ps_acc, "s": ps_s, "o": ps_o, "lb": ps_lb}

                def psum(tag, shape=(P, TC), dtype=F32):
                    return _pools[tag].tile(list(shape), dtype, tag=tag, name=tag)

                def finish_rope(s, t1, jsl):
                    # s <- s*cos + rotate_half(s)*sin; t1 = s*cos precomputed
                    pr = psum("s")
                    nc.tensor.matmul(pr[:], rmat_sb[:], s, start=True, stop=True)
                    nc.vector.tensor_mul(out=s, in0=pr[:], in1=sin_sb[:, jsl])
                    nc.vector.tensor_add(out=s, in0=s, in1=t1[:])

                for j in range(NJ):
                    jsl = slice(j * TC, (j + 1) * TC)
                    # ---- A_j: projections of t-chunk j + RoPE + V transpose.
                    # Chain order V,K,Q0..Q3; each chain's RoPE is emitted one
                    # chain later so its eviction + cos-mul hide under matmuls.
                    if j == 0:
                        xcq = xcq0
                    else:
                        xcq = [load_x_quarter(j, q) for q in range(4)]
                    qt = qk.tile([HD, G, TC], F32R, tag="qt")
                    vt = vtp.tile([HD, TC], F32R, tag="vt")
                    rope_q = []
                    for a in range(6):
                        acc = psum("acc")
                        for dt in range(DT):
                            if a == 0:
                                lhsT = wv_sb[:, dt]
                            elif a == 1:
                                lhsT = wk_sb[:, dt]
                            else:
                                h = a - 2
                                lhsT = wq_sb[:, dt, h * HD:(h + 1) * HD]
                            nc.tensor.matmul(acc[:], lhsT, xcq[dt // 4][:, dt % 4],
                                             start=(dt == 0), stop=(dt == DT - 1))
                        if a == 0:
                            nc.scalar.copy(vt[:], acc[:])
                        else:
                            s = kt_sb[:, jsl] if a == 1 else qt[:, a - 2]
                            nc.scalar.copy(s, acc[:])
                            t1 = rtmp.tile([HD, TC], F32R, tag="t1")
                            nc.vector.tensor_mul(out=t1[:], in0=s,
                                                 in1=cos_sb[:, jsl])
                            rope_q.append((s, t1))
                        if a == 2:
                            for tt in range(NJ):
                                pvt = psum("s", (P, P), F32R)
                                nc.tensor.transpose(pvt[:], vt[:, tt * P:(tt + 1) * P],
                                                    iden_sb[:])
                                nc.vector.tensor_copy(v_sb[:, NJ * j + tt], pvt[:])
                        if len(rope_q) >= 2:
                            finish_rope(*rope_q.pop(0), jsl)
                    while rope_q:
                        finish_rope(*rope_q.pop(0), jsl)

                    # ---- B_j: attention for q-block j, all 4 heads. Diagonal
                    # k-tiles (m = kt-4j >= 0) only compute columns >= off:
                    # earlier columns are fully causal-masked. m=3 uses off=256
                    # (not 384) to keep fp32r matmuls at free-dim >= 256.
                    nk = 4 * (j + 1)
                    OFFS = {0: 0, 1: 128, 2: 256, 3: 256}
                    DEPTH = 3  # exp/mask run three S-tiles ahead of sum/PV
                    po = {}
                    pl = {}
                    pipe = []

                    def finalize(h):
                        rinv = sml.tile([1, TC], F32R, tag="rinv")
                        with nc.allow_low_precision(reason="softmax recip f32r"):
                            nc.vector.reciprocal(rinv[:], pl[h][:])
                        binv = sml.tile([P, TC], F32R, tag="binv")
                        nc.gpsimd.partition_broadcast(binv[:], rinv[:])
                        nc.vector.tensor_mul(out=otn[:, h, jsl], in0=po[h][:],
                                             in1=binv[:])

                    def drain():
                        ppt, ph, pkt, qs = pipe.pop(0)
                        nc.tensor.matmul(pl[ph][:, qs], onek_sb[:], ppt[:, qs],
                                         start=(pkt == 0), stop=(pkt == nk - 1))
                        nc.tensor.matmul(po[ph][:, qs], v_sb[:, pkt], ppt[:, qs],
                                         start=(pkt == 0), stop=(pkt == nk - 1))
                        if pkt == nk - 1:
                            finalize(ph)

                    for h in range(G):
                        po[h] = psum("o")
                        pl[h] = psum("lb", (1, TC))
                        for kt in range(nk):
                            m = kt - 4 * j
                            off = 0 if m < 0 else OFFS[m]
                            qs = slice(off, TC)
                            pss = psum("s")
                            nc.tensor.matmul(pss[:, qs], kt_sb[:, kt * P:(kt + 1) * P],
                                             qt[:, h, qs], start=True, stop=True)
                            pt = wk.tile([P, TC], F32R, tag="pt")
                            nc.scalar.activation(pt[:, qs], pss[:, qs], EXP,
                                                 scale=SCALE)
                            if m >= 0:
                                ssl = slice(off, TC if m == 3 else off + P)
                                nc.vector.tensor_mul(out=pt[:, ssl], in0=pt[:, ssl],
                                                     in1=mask_sb[:, m, ssl])
                            pipe.append((pt, h, kt, qs))
                            if len(pipe) > DEPTH:
                                drain()
                    while pipe:
                        drain()

            # ---- C: output projection, yT = woT.T @ otn (transposed partial).
            # Opens after the A/B pools close: wo_sb lands on freed addresses,
            # so its DMA starts once A_3 releases the weights and hides under
            # B_3. 4-bank psum tiles give 1 MiB output DMAs.
            with (
                tc.tile_pool(name="wo", bufs=1) as wop,
                tc.tile_pool(name="yout", bufs=2) as yop,
                tc.tile_pool(name="psc", bufs=2, space="PSUM") as psc,
            ):
                wo_sb = wop.tile([P, G, D], F32R, tag="wo")
                woT_r = woT.rearrange("(g p) d -> p g d", p=P)
                for g in range(G):
                    nc.sync.dma_start(wo_sb[:, g], woT_r[:, g])
                for dt in range(DT):
                    py = psc.tile([P, NJ * TC], F32, tag="y", name="py")
                    for tj in range(NJ):
                        tsl = slice(tj * TC, (tj + 1) * TC)
                        for g in range(G):
                            nc.tensor.matmul(py[:, tsl],
                                             wo_sb[:, g, dt * P:(dt + 1) * P],
                                             otn[:, g, tsl],
                                             start=(g == 0), stop=(g == G - 1))
                    y_sb = yop.tile([P, NJ * TC], F32, tag="ysb")
                    nc.scalar.copy(y_sb[:], py[:])
                    nc.sync.dma_start(yT[dt * P:(dt + 1) * P, :], y_sb[:])

    nc.compile()
    return nc


def _host_shards(inputs):
    x = np.ascontiguousarray(np.asarray(inputs["x"], dtype=np.float32))
    cos = np.asarray(inputs["cos"], dtype=np.float32)
    sin = np.asarray(inputs["sin"], dtype=np.float32)
    Wq = np.asarray(inputs["Wq"], dtype=np.float32)
    Wk = np.asarray(inputs["Wk"], dtype=np.float32)
    Wv = np.asarray(inputs["Wv"], dtype=np.float32)
    Wo = np.asarray(inputs["Wo"], dtype=np.float32)

    cosT = np.ascontiguousarray(cos.T)
    sinT = np.ascontiguousarray(sin.T)
    rmat = np.zeros((HD, HD), np.float32)
    half = HD // 2
    for i in range(half):
        rmat[i + half, i] = -1.0     # out[m<64] = -q[m+64]
        rmat[i, i + half] = 1.0      # out[m>=64] = q[m-64]
    iden = np.eye(P, dtype=np.float32)
    kk = np.arange(P)[:, None, None]
    mm = np.arange(G)[None, :, None]
    qq = np.arange(TC)[None, None, :]
    masks = (qq >= kk + P * mm).astype(np.float32)
    ones_k = np.ones((P, 1), np.float32)

    xTs = [np.ascontiguousarray(x[b].T) for b in range(B)]
    wqTs = [np.ascontiguousarray(Wq[kv * EQ:(kv + 1) * EQ].T) for kv in range(HKV)]
    wkTs = [np.ascontiguousarray(Wk[kv * HD:(kv + 1) * HD].T) for kv in range(HKV)]
    wvTs = [np.ascontiguousarray(Wv[kv * HD:(kv + 1) * HD].T) for kv in range(HKV)]
    woTs = [np.ascontiguousarray(Wo[:, kv * EQ:(kv + 1) * EQ].T) for kv in range(HKV)]

    in_maps = []
    for c in range(8):
        b, kv = divmod(c, HKV)
        in_maps.append({
            "xT": xTs[b], "wqT": wqTs[kv], "wkT": wkTs[kv], "wvT": wvTs[kv],
            "woT": woTs[kv], "cosT": cosT, "sinT": sinT, "rmat": rmat,
            "iden": iden, "masks": masks, "ones_k": ones_k,
        })
    return in_maps


def get_nc():
    if "nc" not in _CACHE:
        _CACHE["nc"] = _build()
    return _CACHE["nc"]


def run(inputs, **kw):
    nc = get_nc()
    in_maps = _host_shards(inputs)
    res = run_bass_kernel_spmd(nc, in_maps, core_ids=list(range(8)), **kw)
    out = np.zeros((B, T, D), np.float32)
    for c in range(8):
        b = c // HKV
        out[b] += res.results[c]["yT"].T
    return out, res


def kernel(**inputs) -> np.ndarray:
    out, _ = run(inputs)
    return out


# revision 26
# speedup vs baseline: 1.0086x; 1.0086x over previous
"""GQA attention kernel for 8 TRN2 NeuronCores (Bass/Tile, SPMD).

Sharding: core c -> (batch b = c // 4, kv-head kv = c % 4). Each core computes
the 4 query heads of its kv group for its batch and a partial (transposed)
output projection; the host sums the 4 partials per batch.

All device matmuls run in float32r (fp32 bits, full-rate PE path at free-dim
>= 256). Attention is computed in transposed layout throughout:
  QT/KT [hd, t]  ->  S.T [k, q] psum  ->  exp on ACT  ->  P.T [k, q]
  O.T [hd, q] = V[k, hd].T-accumulated PV matmuls
Softmax denominators come from a ones-vector matmul accumulated alongside PV;
normalization is a gpsimd partition-broadcast of 1/l followed by a DVE mul.
RoPE's rotate-half is a +-1 permutation matmul on the hd (partition) axis.
"""

import os
import sys

import numpy as np

for _p in ("/opt/trn_rl_repo", "/root/.axon_site/_ro/trn_rl_repo"):
    if os.path.isdir(_p) and _p not in sys.path:
        sys.path.insert(0, _p)

import concourse.bass as bass  # noqa: E402
import concourse.mybir as mybir  # noqa: E402
from concourse import bacc  # noqa: E402
from concourse.tile import TileContext  # noqa: E402
from concourse.bass_utils import run_bass_kernel_spmd  # noqa: E402

B, T, D = 2, 2048, 2048
H, HKV, HD = 16, 4, 128
G = H // HKV            # query heads per kv head (= per core)
EQ = G * HD             # 512: query-projection rows per core
P = 128
TC = 512                # t-chunk (free dim of every matmul)
NJ = T // TC            # 4 chunks
DT = D // P             # 16 contraction tiles
SCALE = 1.0 / float(np.sqrt(HD))

F32 = mybir.dt.float32
F32R = mybir.dt.float32r
EXP = mybir.ActivationFunctionType.Exp

_CACHE = {}


def _build():
    nc = bacc.Bacc("TRN2", target_bir_lowering=False, debug=False)

    # All inputs arrive pre-transposed into SBUF layout (partition dim first,
    # contiguous per partition) so every DMA runs at full descriptor rate.
    xT = nc.declare_dram_parameter("xT", [P, NJ, 4, 4, TC], F32R, isOutput=False)
    wqT = nc.declare_dram_parameter("wqT", [P, DT, EQ], F32R, isOutput=False)
    wkT = nc.declare_dram_parameter("wkT", [P, DT, HD], F32R, isOutput=False)
    wvT = nc.declare_dram_parameter("wvT", [P, DT, HD], F32R, isOutput=False)
    woT = nc.declare_dram_parameter("woT", [P, G, D], F32R, isOutput=False)
    cosT = nc.declare_dram_parameter("cosT", [HD, T], F32R, isOutput=False)
    sinT = nc.declare_dram_parameter("sinT", [HD, T], F32R, isOutput=False)
    rmat = nc.declare_dram_parameter("rmat", [HD, HD], F32R, isOutput=False)
    iden = nc.declare_dram_parameter("iden", [P, P], F32R, isOutput=False)
    masks = nc.declare_dram_parameter("masks", [P, G, TC], F32R, isOutput=False)
    ones_k = nc.declare_dram_parameter("ones_k", [P, 1], F32R, isOutput=False)
    yT = nc.declare_dram_parameter("yT", [D, T], F32, isOutput=True)

    with TileContext(nc) as tc:
        with (
            tc.tile_pool(name="const", bufs=1) as cst,
            tc.tile_pool(name="kv", bufs=1) as kvp,
            tc.tile_pool(name="ot", bufs=1) as otp,
        ):
            # Constants ride the gpsimd SWDGE ring so they don't delay the
            # weight/x loads on the two HWDGE rings.
            cos_sb = cst.tile([HD, T], F32R, tag="cos")
            sin_sb = cst.tile([HD, T], F32R, tag="sin")
            rmat_sb = cst.tile([HD, HD], F32R, tag="rmat")
            iden_sb = cst.tile([P, P], F32R, tag="iden")
            mask_sb = cst.tile([P, G, TC], F32R, tag="mask")
            onek_sb = cst.tile([P, 1], F32R, tag="onek")
            nc.gpsimd.dma_start(cos_sb[:], cosT[:])
            nc.gpsimd.dma_start(sin_sb[:], sinT[:])
            nc.gpsimd.dma_start(rmat_sb[:], rmat[:])
            nc.gpsimd.dma_start(iden_sb[:], iden[:])
            nc.gpsimd.dma_start(mask_sb[:], masks[:])
            nc.gpsimd.dma_start(onek_sb[:], ones_k[:])

            kt_sb = kvp.tile([HD, T], F32R, tag="kt")
            v_sb = kvp.tile([P, DT, HD], F32R, tag="v")
            otn = otp.tile([HD, G, T], F32R, tag="otn")

            with (
                tc.tile_pool(name="wts", bufs=1) as wts,
                tc.tile_pool(name="xs", bufs=1) as xs,
                tc.tile_pool(name="qk", bufs=2) as qk,
                tc.tile_pool(name="work", bufs=5) as wk,
                tc.tile_pool(name="rtmp", bufs=2) as rtmp,
                tc.tile_pool(name="vt", bufs=2) as vtp,
                tc.tile_pool(name="small", bufs=2) as sml,
                tc.tile_pool(name="ps_acc", bufs=2, space="PSUM") as ps_acc,
                tc.tile_pool(name="ps_s", bufs=2, space="PSUM") as ps_s,
                tc.tile_pool(name="ps_o", bufs=2, space="PSUM") as ps_o,
                tc.tile_pool(name="ps_lb", bufs=2, space="PSUM") as ps_lb,
            ):
                # Weights ride the scalar HWDGE ring; x-chunks ride the sync
                # ring. wq is loaded in dt-quarters so the Q chains can start
                # before the full 4 MiB lands. Chain order (V,K,Q0..Q3)
                # matches DMA arrival order on each ring.
                wq_sb = wts.tile([P, DT, EQ], F32R, tag="wq")
                wk_sb = wts.tile([P, DT, HD], F32R, tag="wk")
                wv_sb = wts.tile([P, DT, HD], F32R, tag="wv")
                def load_x_quarter(j, q):
                    xq = xs.tile([P, 4, TC], F32R, tag=f"xc{q}", name=f"xc{q}")
                    nc.sync.dma_start(xq[:], xT[:, j, q])
                    return xq

                # One HWDGE ring executes dma_starts FIFO, so emit the loads
                # in the exact order phase A consumes them: V weights, first x
                # quarter, K weights, then alternating x quarters / wq slices.
                nc.sync.dma_start(wv_sb[:], wvT[:])
                xcq0 = [load_x_quarter(0, 0)]
                nc.sync.dma_start(wk_sb[:], wkT[:])
                xcq0.append(load_x_quarter(0, 1))
                for q in range(4):
                    nc.sync.dma_start(wq_sb[:, 4 * q:4 * q + 4],
                                      wqT[:, 4 * q:4 * q + 4])
                    if q < 2:
                        xcq0.append(load_x_quarter(0, q + 2))

                _pools = {"acc": ps_acc, "s": ps_s, "o": ps_o, "lb": ps_lb}

                def psum(tag, shape=(P, TC), dtype=F32):
                    return _pools[tag].tile(list(shape), dtype, tag=tag, name=tag)

                def finish_rope(s, t1, jsl):
                    # s <- s*cos + rotate_half(s)*sin; t1 = s*cos precomputed
                    pr = psum("s")
                    nc.tensor.matmul(pr[:], rmat_sb[:], s, start=True, stop=True)
                    nc.vector.tensor_mul(out=s, in0=pr[:], in1=sin_sb[:, jsl])
                    nc.vector.tensor_add(out=s, in0=s, in1=t1[:])

                for j in range(NJ):
                    jsl = slice(j * TC, (j + 1) * TC)
                    # ---- A_j: projections of t-chunk j + RoPE + V transpose.
                    # Chain order V,K,Q0..Q3; each chain's RoPE is emitted one
                    # chain later so its eviction + cos-mul hide under matmuls.
                    if j == 0:
                        xcq = xcq0
                    else:
                        xcq = [load_x_quarter(j, q) for q in range(4)]
                    qt = qk.tile([HD, G, TC], F32R, tag="qt")
                    vt = vtp.tile([HD, TC], F32R, tag="vt")
                    rope_q = []
                    for a in range(6):
                        acc = psum("acc")
                        for dt in range(DT):
                            if a == 0:
                                lhsT = wv_sb[:, dt]
                            elif a == 1:
                                lhsT = wk_sb[:, dt]
                            else:
                                h = a - 2
                                lhsT = wq_sb[:, dt, h * HD:(h + 1) * HD]
                            nc.tensor.matmul(acc[:], lhsT, xcq[dt // 4][:, dt % 4],
                                             start=(dt == 0), stop=(dt == DT - 1))
                        if a == 0:
                            nc.scalar.copy(vt[:], acc[:])
                        else:
                            s = kt_sb[:, jsl] if a == 1 else qt[:, a - 2]
                            nc.scalar.copy(s, acc[:])
                            t1 = rtmp.tile([HD, TC], F32R, tag="t1")
                            nc.vector.tensor_mul(out=t1[:], in0=s,
                                                 in1=cos_sb[:, jsl])
                            rope_q.append((s, t1))
                        if a == 2:
                            for tt in range(NJ):
                                pvt = psum("s", (P, P), F32R)
                                nc.tensor.transpose(pvt[:], vt[:, tt * P:(tt + 1) * P],
                                                    iden_sb[:])
                                nc.vector.tensor_copy(v_sb[:, NJ * j + tt], pvt[:])
                        if len(rope_q) >= 2:
                            finish_rope(*rope_q.pop(0), jsl)
                    while rope_q:
                        finish_rope(*rope_q.pop(0), jsl)

                    # ---- B_j: attention for q-block j, all 4 heads. Diagonal
                    # k-tiles (m = kt-4j >= 0) only compute columns >= off:
                    # earlier columns are fully causal-masked. m=3 uses off=256
                    # (not 384) to keep fp32r matmuls at free-dim >= 256.
                    nk = 4 * (j + 1)
                    OFFS = {0: 0, 1: 128, 2: 256, 3: 256}
                    DEPTH = 3  # exp/mask run three S-tiles ahead of sum/PV
                    po = {}
                    pl = {}
                    pipe = []

                    def finalize(h):
                        rinv = sml.tile([1, TC], F32R, tag="rinv")
                        with nc.allow_low_precision(reason="softmax recip f32r"):
                            nc.vector.reciprocal(rinv[:], pl[h][:])
                        binv = sml.tile([P, TC], F32R, tag="binv")
                        nc.gpsimd.partition_broadcast(binv[:], rinv[:])
                        nc.vector.tensor_mul(out=otn[:, h, jsl], in0=po[h][:],
                                             in1=binv[:])

                    def drain():
                        ppt, ph, pkt, qs = pipe.pop(0)
                        nc.tensor.matmul(pl[ph][:, qs], onek_sb[:], ppt[:, qs],
                                         start=(pkt == 0), stop=(pkt == nk - 1))
                        nc.tensor.matmul(po[ph][:, qs], v_sb[:, pkt], ppt[:, qs],
                                         start=(pkt == 0), stop=(pkt == nk - 1))
                        if pkt == nk - 1:
                            finalize(ph)

                    for h in range(G):
                        po[h] = psum("o")
                        pl[h] = psum("lb", (1, TC))
                        for kt in range(nk):
                            m = kt - 4 * j
                            off = 0 if m < 0 else OFFS[m]
                            qs = slice(off, TC)
                            pss = psum("s")
                            nc.tensor.matmul(pss[:, qs], kt_sb[:, kt * P:(kt + 1) * P],
                                             qt[:, h, qs], start=True, stop=True)
                            pt = wk.tile([P, TC], F32R, tag="pt")
                            nc.scalar.activation(pt[:, qs], pss[:, qs], EXP,
                                                 scale=SCALE)
                            if m >= 0:
                                ssl = slice(off, TC if m == 3 else off + P)
                                nc.vector.tensor_mul(out=pt[:, ssl], in0=pt[:, ssl],
                                                     in1=mask_sb[:, m, ssl])
                            pipe.append((pt, h, kt, qs))
                            if len(pipe) > DEPTH:
                                drain()
                    while pipe:
                        drain()

            # ---- C: output projection, yT = woT.T @ otn (transposed partial).
            # Opens after the A/B pools close: wo_sb lands on freed addresses,
            # so its DMA starts once A_3 releases the weights and hides under
            # B_3. 4-bank psum tiles give 1 MiB output DMAs.
            with (
                tc.tile_pool(name="wo", bufs=1) as wop,
                tc.tile_pool(name="yout", bufs=2) as yop,
                tc.tile_pool(name="psc", bufs=2, space="PSUM") as psc,
            ):
                wo_sb = wop.tile([P, G, D], F32R, tag="wo")
                for g in range(G):
                    nc.sync.dma_start(wo_sb[:, g], woT[:, g])
                for dt in range(DT):
                    py = psc.tile([P, NJ * TC], F32, tag="y", name="py")
                    for tj in range(NJ):
                        tsl = slice(tj * TC, (tj + 1) * TC)
                        for g in range(G):
                            nc.tensor.matmul(py[:, tsl],
                                             wo_sb[:, g, dt * P:(dt + 1) * P],
                                             otn[:, g, tsl],
                                             start=(g == 0), stop=(g == G - 1))
                    y_sb = yop.tile([P, NJ * TC], F32, tag="ysb")
                    nc.scalar.copy(y_sb[:], py[:])
                    nc.sync.dma_start(yT[dt * P:(dt + 1) * P, :], y_sb[:])

    nc.compile()
    return nc


def _host_shards(inputs):
    x = np.ascontiguousarray(np.asarray(inputs["x"], dtype=np.float32))
    cos = np.asarray(inputs["cos"], dtype=np.float32)
    sin = np.asarray(inputs["sin"], dtype=np.float32)
    Wq = np.asarray(inputs["Wq"], dtype=np.float32)
    Wk = np.asarray(inputs["Wk"], dtype=np.float32)
    Wv = np.asarray(inputs["Wv"], dtype=np.float32)
    Wo = np.asarray(inputs["Wo"], dtype=np.float32)

    cosT = np.ascontiguousarray(cos.T)
    sinT = np.ascontiguousarray(sin.T)
    rmat = np.zeros((HD, HD), np.float32)
    half = HD // 2
    for i in range(half):
        rmat[i + half, i] = -1.0     # out[m<64] = -q[m+64]
        rmat[i, i + half] = 1.0      # out[m>=64] = q[m-64]
    iden = np.eye(P, dtype=np.float32)
    kk = np.arange(P)[:, None, None]
    mm = np.arange(G)[None, :, None]
    qq = np.arange(TC)[None, None, :]
    masks = (qq >= kk + P * mm).astype(np.float32)
    ones_k = np.ones((P, 1), np.float32)

    def to_sbuf_layout(wT, cols):
        # [D_contract, cols] -> [P, D_contract//P, cols], partition dim first
        return np.ascontiguousarray(
            wT.reshape(-1, P, cols).transpose(1, 0, 2))

    # x[b].T is [d, t]; device layout [p, j, q, dtq, t'] with d = (4q+dtq)*P+p
    # and t = j*TC + t' makes each (j, q) quarter-load fully contiguous.
    xTs = [np.ascontiguousarray(
        x[b].T.reshape(4, 4, P, NJ, TC).transpose(2, 3, 0, 1, 4))
        for b in range(B)]
    wqTs = [to_sbuf_layout(Wq[kv * EQ:(kv + 1) * EQ].T, EQ) for kv in range(HKV)]
    wkTs = [to_sbuf_layout(Wk[kv * HD:(kv + 1) * HD].T, HD) for kv in range(HKV)]
    wvTs = [to_sbuf_layout(Wv[kv * HD:(kv + 1) * HD].T, HD) for kv in range(HKV)]
    woTs = [to_sbuf_layout(Wo[:, kv * EQ:(kv + 1) * EQ].T, D) for kv in range(HKV)]

    in_maps = []
    for c in range(8):
        b, kv = divmod(c, HKV)
        in_maps.append({
            "xT": xTs[b], "wqT": wqTs[kv], "wkT": wkTs[kv], "wvT": wvTs[kv],
            "woT": woTs[kv], "cosT": cosT, "sinT": sinT, "rmat": rmat,
            "iden": iden, "masks": masks, "ones_k": ones_k,
        })
    return in_maps


def get_nc():
    if "nc" not in _CACHE:
        _CACHE["nc"] = _build()
    return _CACHE["nc"]


def run(inputs, **kw):
    nc = get_nc()
    in_maps = _host_shards(inputs)
    res = run_bass_kernel_spmd(nc, in_maps, core_ids=list(range(8)), **kw)
    out = np.zeros((B, T, D), np.float32)
    for c in range(8):
        b = c // HKV
        out[b] += res.results[c]["yT"].T
    return out, res


def kernel(**inputs) -> np.ndarray:
    out, _ = run(inputs)
    return out
